# revision 5
# baseline (speedup 1.0000x reference)
"""AttentionPooling segment-reduce kernel for 8 Trainium2 NeuronCores.

Math (reference):
    k = x @ key_w.T + key_b            # [N, 256] -> heads [N, 4, 64]
    v = x @ value_w.T + value_b
    attn   = einsum('hd,nhd->nh', query, k) * SCALE
    w      = exp(attn)
    wsum   = segment_sum(w)[batch]
    out[b] = segment_sum(w/(wsum+EPS) * v)

Algebraic restructuring (exact):
    attn[n,h] = qt[:,h] . x[n] + sc[h],  qt = SCALE*(key_w^T q per head),
                                         sc = SCALE*(q . key_b per head)
    w = exp(attn) = g[h]*wt[n,h],  wt = exp(qt . x),  g = exp(sc)
    v' = x @ value_w.T                 (bias deferred to segment level)
    St[b,f] = sum_{n in b} wt[n,h(f)] v'[n,f];  dt[b,h] = sum_{n in b} wt[n,h]
    out[b,f] = (St[b,f] + dt[b,h]*value_b[f]) / (dt[b,h] + EPS/g[h])

Device mapping: core c owns segments [c*512,(c+1)*512) in 4 windows of 128
segments.  Each window's 128 segments are host-partitioned into 4 GROUPS of
32 segments balanced by node count; group g's segments map to PSUM rows
[32g, 32g+32) (the host un-permutes output rows afterwards).  Per tile of
128 nodes (each tile belongs to one group):
 - PE proj: psum[nodes,260] = xT.T @ [Wv^T | qt]  (2 fused matmuls).
 - ACT: exp of 4 attn cols -> wt;  plain copy of v' heads 2,3 to SBUF.
 - DVE: u01 = psum[:,0:128] * wt[0:2] (heads 0,1 fused multiply+drain).
 - GPS: ohw[h] = onehot * wt[h] for heads 2,3 (weighted one-hot builds).
 - PE segred: per tile 3 col-tiled matmuls (M=32, tile_position=(0,32g)):
     oh x [wt|u01] (N=132), ohw2 x v2' (N=64), ohw3 x v3' (N=64), into
   psum_s rows [32g,32g+32), cols [dt 0:4 | St 4:260].
Rounds process one tile from each of the 4 groups; the 12 segred MMs are
issued kind-major (a,a,a,a,b,b,b,b,c,c,c,c) so the 4 column-groups of the
PE array stream concurrently on separate XBUSes.  Round r's segred is
issued after round r+1's projection so the PE never waits on the drains.
One-hot node->segment matrices (32 wide) are precomputed on the host and
streamed alongside x^T.  Window epilogue (DVE): out = (St + dt*bv) /
(dt + eps/g), DMA to the core's (permuted) output rows.
"""

from contextlib import ExitStack

import numpy as np

N = 262144
DIM = 256
H = 4
HD = 64
B = 4096
SCALE = HD ** (-0.5)
EPS = 1e-8

NCORES = 8
SEGS_PER_CORE = B // NCORES          # 512
WPC = 4                              # windows per core
WSEG = SEGS_PER_CORE // WPC          # 128 segments per window
NG = 4                               # column-groups per window
GSEG = WSEG // NG                    # 32 segments per group
CHUNKT = 8                           # tiles per DMA chunk
TBLK = 288                           # pk cols per tile: x_lo|x_hi|oh

TRACE = False                        # test harness can flip for profiling
LAST_RESULT = None

_cache = {}


def _segred(nc, psum_s, args, start, stop):
    """Issue one round's segment-reduce matmuls, kind-major so the four
    32-wide column groups run concurrently.  PSUM start=True clears the
    has_written bits of the written partition rows across ALL columns of
    the bank, so only the FIRST matmul per group-rows (kind a/A) may carry
    start=True; kinds b,c write into the freshly cleared region with
    start=False (has_written=0 there -> store semantics)."""
    for g, oh, u01, u23, ohw, b in args:
        if u23 is None:                           # kind A: all heads, N=260
            nc.tensor.matmul(psum_s[32 * g:32 * g + 32, 0:260], oh,
                             u01[:, b, :], start=start, stop=stop,
                             tile_position=(0, 32 * g))
        else:                                     # kind a: heads 0,1, N=132
            nc.tensor.matmul(psum_s[32 * g:32 * g + 32, 0:132], oh,
                             u01[:, b, :], start=start, stop=stop,
                             tile_position=(0, 32 * g))
    for g, oh, u01, u23, ohw, b in args:          # kind b: head 2, N=64
        if u23 is None:
            continue
        nc.tensor.matmul(psum_s[32 * g:32 * g + 32, 132:196],
                         ohw[:, b, 0, :], u23[:, b, 0:64],
                         start=False, stop=stop, tile_position=(0, 32 * g))
    for g, oh, u01, u23, ohw, b in args:          # kind c: head 3, N=64
        if u23 is None:
            continue
        nc.tensor.matmul(psum_s[32 * g:32 * g + 32, 196:260],
                         ohw[:, b, 1, :], u23[:, b, 64:128],
                         start=False, stop=stop, tile_position=(0, 32 * g))


def _build(tq: int):
    """Build + compile the SPMD program for tq tiles per (window, group)."""
    import concourse.tile as tile
    from concourse import bacc, mybir

    F32 = mybir.dt.float32
    F16 = mybir.dt.float16
    Alu = mybir.AluOpType
    Act = mybir.ActivationFunctionType

    T = WPC * NG * tq                # total tiles per core

    nc = bacc.Bacc("TRN2", target_bir_lowering=False, debug=False,
                   num_devices=NCORES)

    pk_d = nc.dram_tensor("pk", [128, TBLK * T], F16, kind="ExternalInput").ap()
    wq_d = nc.dram_tensor("wq", [128, 520], F16, kind="ExternalInput").ap()
    cst_d = nc.dram_tensor("cst", [128, 260], F32, kind="ExternalInput").ap()
    out_d = nc.dram_tensor("out", [SEGS_PER_CORE, 256], F32,
                           kind="ExternalOutput").ap()

    with tile.TileContext(nc, pool_alloc_mode="queue") as tc, \
            ExitStack() as ctx:
        consts = ctx.enter_context(tc.tile_pool(name="consts", bufs=1))
        xin = ctx.enter_context(tc.tile_pool(name="xin", bufs=4))
        u01p = ctx.enter_context(tc.tile_pool(name="u01p", bufs=4))
        u23p = ctx.enter_context(tc.tile_pool(name="u23p", bufs=4))
        ohwp = ctx.enter_context(tc.tile_pool(name="ohwp", bufs=4))
        fxp = ctx.enter_context(tc.tile_pool(name="fxp", bufs=2))
        pp = ctx.enter_context(tc.tile_pool(name="pp", bufs=3, space="PSUM"))
        sp = ctx.enter_context(tc.tile_pool(name="sp", bufs=2, space="PSUM"))

        # PE warm-up: dummy matmuls on zeros, issued with no DMA dependency
        # so they run during the initial input-chunk DMA wait and flip the
        # HAM clock gate to 2.4 GHz before real work arrives.
        wtile = consts.tile([128, 128], F16, tag="wtile")
        nc.vector.memset(wtile[:], 0.0)
        wpsum = pp.tile([128, 2 * 512], F32, tag="pp")
        for _ in range(34):
            nc.tensor.matmul(wpsum[:, 0:128], wtile[:], wtile[:],
                             start=True, stop=True)

        wqpk = consts.tile([128, 520], F16, tag="wqpk")
        cst = consts.tile([128, 260], F32, tag="cst")
        nc.sync.dma_start(wqpk[:], wq_d)
        cst_loaded = False
        wq0 = wqpk[:, 0:260]
        wq1 = wqpk[:, 260:520]
        bvrep = cst[:, 0:256]
        epsg = cst[:, 256:260]

        pkt = None
        for w in range(WPC):
            psum_s = sp.tile([128, 260], F32, tag="ps")
            pend = None
            for r in range(tq):
                args = []          # per-tile segred operands, this round
                for half in range(2):
                    psum4 = pp.tile([128, 2 * 512], F32, tag="pp")
                    ohv = []
                    o0 = None
                    for b in range(2):
                        g = half * 2 + b
                        t = (w * tq + r) * NG + g       # linear tile index
                        if t % CHUNKT == 0:
                            cw = min(CHUNKT, T - t)
                            pkt = xin.tile([128, TBLK * CHUNKT], F16,
                                           tag="pkt")
                            nc.sync.dma_start(
                                pkt[:, 0:TBLK * cw],
                                pk_d[:, TBLK * t:TBLK * (t + cw)])
                            if not cst_loaded:
                                cst_loaded = True
                                nc.sync.dma_start(cst[:], cst_d)
                        o = (t % CHUNKT) * TBLK
                        if b == 0:
                            o0 = o
                        ps = psum4[:, b * 512:b * 512 + 260]
                        nc.tensor.matmul(ps, pkt[:, o:o + 128], wq0,
                                         start=True, stop=False)
                        nc.tensor.matmul(ps, pkt[:, o + 128:o + 256], wq1,
                                         start=False, stop=True)
                        ohv.append(pkt[:, o + 256:o + 288])

                    p3 = psum4[:].rearrange("p (b c) -> p b c", c=512)
                    if half == 0:
                        # half A: DVE fuses heads 0,1; ACT copies heads 2,3
                        # (wt-scaling folded into PE stationaries via GPS).
                        u01 = u01p.tile([128, 2, 132], F16, tag="u01a")
                        u23 = u23p.tile([128, 2, 128], F16, tag="u23")
                        ohw = ohwp.tile([128, 2, 2, 32], F16, tag="ohw")
                        nc.scalar.activation(u01[:, :, 0:4],
                                             p3[:, :, 256:260], Act.Exp)
                        in0 = (p3[:, :, 0:128]
                               .rearrange("p b (h d) -> p b h d", h=2))
                        in1 = (u01[:, :, 0:2].unsqueeze(3)
                               .broadcast_to([128, 2, 2, 64]))
                        o4 = (u01[:, :, 4:132]
                              .rearrange("p b (h d) -> p b h d", h=2))
                        nc.vector.tensor_tensor(o4, in0, in1, Alu.mult)
                        nc.scalar.activation(u23[:, :, :], p3[:, :, 128:256],
                                             Act.Copy)
                        # weighted one-hots for heads 2,3 on GpSimd: the two
                        # tiles of this half are adjacent TBLK pkt blocks.
                        gi0 = (pkt[:, o0:o0 + 2 * TBLK]
                               .rearrange("p (t c) -> p t c", t=2)
                               [:, :, 256:288]
                               .unsqueeze(2).broadcast_to([128, 2, 2, 32]))
                        gi1 = (u01[:, :, 2:4].unsqueeze(3)
                               .broadcast_to([128, 2, 2, 32]))
                        nc.gpsimd.tensor_tensor(ohw[:], gi0, gi1, Alu.mult)
                        for b in range(2):
                            args.append((half * 2 + b, ohv[b], u01, u23,
                                         ohw, b))
                    else:
                        # half B: DVE fuses all 4 heads in one op.
                        u01 = u01p.tile([128, 2, 260], F16, tag="u01b")
                        nc.scalar.activation(u01[:, :, 0:4],
                                             p3[:, :, 256:260], Act.Exp)
                        in0 = (p3[:, :, 0:256]
                               .rearrange("p b (h d) -> p b h d", h=4))
                        in1 = (u01[:, :, 0:4].unsqueeze(3)
                               .broadcast_to([128, 2, 4, 64]))
                        o4 = (u01[:, :, 4:260]
                              .rearrange("p b (h d) -> p b h d", h=4))
                        nc.vector.tensor_tensor(o4, in0, in1, Alu.mult)
                        for b in range(2):
                            args.append((half * 2 + b, ohv[b], u01, None,
                                         None, b))

                if pend is not None:
                    _segred(nc, psum_s, pend, start=(r == 1), stop=False)
                pend = args
            _segred(nc, psum_s, pend, start=(tq == 1), stop=True)

            # ---- window epilogue ----
            dsum = fxp.tile([128, 4], F32, tag="dsum")
            nc.vector.tensor_tensor(dsum[:], psum_s[:, 0:4], epsg, Alu.add)
            rec = fxp.tile([128, 4], F32, tag="rec")
            nc.vector.reciprocal(rec[:], dsum[:])
            t1 = fxp.tile([128, 256], F32, tag="t1")
            bv3 = bvrep.rearrange("p (h d) -> p h d", h=H)
            dt3 = (psum_s[:, 0:4].unsqueeze(2)
                   .broadcast_to([128, H, HD]))
            nc.vector.tensor_tensor(
                t1[:].rearrange("p (h d) -> p h d", h=H), bv3, dt3, Alu.mult)
            t2 = fxp.tile([128, 256], F32, tag="t2")
            nc.vector.tensor_tensor(t2[:], psum_s[:, 4:260], t1[:], Alu.add)
            outt = fxp.tile([128, 256], F32, tag="outt")
            rec3 = rec[:].unsqueeze(2).broadcast_to([128, H, HD])
            nc.vector.tensor_tensor(
                outt[:].rearrange("p (h d) -> p h d", h=H),
                t2[:].rearrange("p (h d) -> p h d", h=H), rec3, Alu.mult)
            nc.sync.dma_start(out_d[w * 128:(w + 1) * 128, :], outt[:])

    nc.compile()
    return nc


def kernel(x, batch, query, key_w, key_b, value_w, value_b):
    global LAST_RESULT
    from concourse.bass_utils import run_bass_kernel_spmd

    x = np.asarray(x, dtype=np.float32)
    batch = np.asarray(batch).astype(np.int64)
    query = np.asarray(query, dtype=np.float32)
    key_w = np.asarray(key_w, dtype=np.float32)
    key_b = np.asarray(key_b, dtype=np.float32)
    value_w = np.asarray(value_w, dtype=np.float32)
    value_b = np.asarray(value_b, dtype=np.float32)

    # ---- host-side planning ----
    counts = np.bincount(batch, minlength=B)
    cum = np.zeros(B + 1, np.int64)
    cum[1:] = np.cumsum(counts)
    nwin = NCORES * WPC                           # 32 windows of 128 segs

    # Balance each window's 128 segments into NG groups of GSEG segments by
    # node count (greedy LPT).  groups[m, g, j] = global segment id mapped
    # to output row  m*WSEG + 32*g + j  (row -> segment permutation).
    groups = np.empty((nwin, NG, GSEG), np.int64)
    gnodes = np.zeros((nwin, NG), np.int64)
    for m in range(nwin):
        segs = np.arange(m * WSEG, (m + 1) * WSEG)
        order = np.argsort(-counts[segs], kind="stable")
        gsum = [0] * NG
        gfill = [0] * NG
        for si in order:
            s = segs[si]
            g = min((gg for gg in range(NG) if gfill[gg] < GSEG),
                    key=lambda gg: gsum[gg])
            groups[m, g, gfill[g]] = s
            gsum[g] += counts[s]
            gfill[g] += 1
        gnodes[m] = gsum

    tq = int(np.max((gnodes + 127) // 128))
    T = WPC * NG * tq

    # Node permutation: sort nodes by their segment's layout position so
    # each (window, group) owns a contiguous run, original order preserved.
    seg_pos = np.empty(B, np.int64)               # seg -> layout rank
    seg_of_row = groups.reshape(-1)               # layout rank -> seg
    seg_pos[seg_of_row] = np.arange(B)
    perm = np.argsort(seg_pos[batch], kind="stable")

    # rank of each segment within its group (one-hot column), per node
    jrank = np.empty(B, np.int64)
    jrank[groups.reshape(-1)] = np.tile(np.arange(GSEG), nwin * NG)
    node_j = jrank[batch[perm]]                   # per permuted node

    # ---- shared constants ----
    wqf = np.zeros((256, 260), np.float32)
    wqf[:, 0:256] = value_w.T
    qt = (key_w.reshape(H, HD, DIM) * query[:, :, None]).sum(axis=1)  # [H,256]
    wqf[:, 256:260] = SCALE * qt.T
    wq = np.concatenate([wqf[0:128], wqf[128:256]],
                        axis=1).astype(np.float16)          # [128, 520]
    sc = SCALE * (query * key_b.reshape(H, HD)).sum(axis=1)           # [H]
    g = np.exp(sc).astype(np.float32)
    cst = np.zeros((128, 260), np.float32)
    cst[:, 0:256] = value_b
    cst[:, 256:260] = EPS / g

    # ---- per-core shards ----
    xT16 = None
    in_maps = []
    gn3 = gnodes.reshape(NCORES, WPC, NG)
    grp_off = np.zeros((NCORES, WPC, NG), np.int64)
    off = 0
    for c in range(NCORES):
        for w in range(WPC):
            for gg in range(NG):
                grp_off[c, w, gg] = off
                off += gn3[c, w, gg]

    for c in range(NCORES):
        pk = np.zeros((128, TBLK * T), np.float16)
        pk4 = pk.reshape(128, T, TBLK)
        for w in range(WPC):
            for gg in range(NG):
                ofs = int(grp_off[c, w, gg])
                L = int(gn3[c, w, gg])
                nodes = perm[ofs:ofs + L]
                ntile = (L + 127) // 128
                xpad = np.zeros((ntile * 128, DIM), np.float16)
                xpad[0:L] = x[nodes]
                xt = xpad.reshape(ntile, 128, DIM).transpose(2, 0, 1)
                blks = (w * tq + np.arange(ntile)) * NG + gg
                pk4[:, blks, 0:128] = xt[0:128]
                pk4[:, blks, 128:256] = xt[128:256]
                jj = node_j[ofs:ofs + L]
                ii = np.arange(L)
                pk4[ii % 128, blks[ii // 128], 256 + jj] = np.float16(1.0)
        in_maps.append({"pk": pk, "wq": wq, "cst": cst})

    if tq not in _cache:
        _cache[tq] = _build(tq)
    nc = _cache[tq]

    res = run_bass_kernel_spmd(nc, in_maps, core_ids=list(range(NCORES)),
                               trace=TRACE)
    LAST_RESULT = res
    out_rows = np.concatenate([r["out"] for r in res.results], axis=0)
    out = np.empty((B, DIM), np.float32)
    out[seg_of_row] = out_rows
    return out
